# revision 29
# baseline (speedup 1.0000x reference)
"""MultiHead GQA (16 q heads / 4 kv heads, E=1024, n=2048, b=1) on 8 TRN2 cores.

Strategy: shard the 2048 query positions across the 8 cores (256 each); every
core computes the full K/V projections for all 2048 key positions (replicating
that small matmul is far cheaper than any collective at this size), runs
attention + layernorm for its own 256 tokens, and writes its 256x1024 slice.

All heavy matmuls run in bf16 (fp32 PSUM accumulation).  The host pre-
transposes and pre-casts the operands so that every matmul contraction
dimension lands on SBUF partitions with no on-device transposes; the only
on-device transposes are small bf16 DMA X-bar transposes of the attention
output (80x128 per chunk) and of the softmax denominators.

The emission order software-pipelines PE: the attention stream is a single
sequence of (pair, s-tile) slot groups; each group's S matmuls are followed
by projection "filler" matmuls and the *previous* group's O/R matmuls, so
the (in-order) PE queue never blocks on the exp that gates O/R.

Load order is tuned so the q-projection (smallest prefix: 1MB) starts the PE
early and warms the HAM clock gate before the S stream begins; the layernorm
rstd is computed as exp(-0.5*ln(var+eps)) so only one activation table set
(natural_log_exp_and_others) is ever loaded.

RoPE in the reference is the identity for b=1 (seq index = batch index = 0,
so cos=1 / sin=0 exactly); it is therefore omitted.
"""

import numpy as np
import ml_dtypes

import concourse.bass as bass
import concourse.bacc as bacc
import concourse.tile as tile
from concourse import mybir
from concourse import bass_utils

F32 = mybir.dt.float32
BF16 = mybir.dt.bfloat16
AF = mybir.ActivationFunctionType
ALU = mybir.AluOpType

N_CORES = 8
E = 1024
QH = 16
KVH = 4
HD = 64
KVE = KVH * HD
SEQ = 2048
T = SEQ // N_CORES   # 256 query tokens per core
ST = SEQ // 128      # 16 key s-tiles
ET = E // 128        # 8 contraction e-tiles
EPS = 1e-5
SCALE = 1.0 / (HD ** 0.5)
TR = 80              # transpose row count (>= HD+1, multiple of 16)

# Head bookkeeping: q-proj channel-tile t packs head EH[t] in partitions 0-63
# and head OH[t] in partitions 64-127.  EH heads use even kv heads (0, 2),
# OH heads use odd kv heads (1, 3), which matches the natural K-proj layout
# (K channel-tile 0 = kv0|kv1, tile 1 = kv2|kv3) with no partition shifts.
EH = [0, 1, 2, 3, 8, 9, 10, 11]
OH = [4, 5, 6, 7, 12, 13, 14, 15]
KV_LO = [EH[2 * p] // 4 for p in range(4)]   # [0, 0, 2, 2]
KV_HI = [OH[2 * p] // 4 for p in range(4)]   # [1, 1, 3, 3]

COLPERM = np.concatenate(
    [np.r_[EH[t] * HD:(EH[t] + 1) * HD, OH[t] * HD:(OH[t] + 1) * HD]
     for t in range(8)])

_CACHE = {}


def _build(apply_gb=True):
    nc = bacc.Bacc("TRN2", target_bir_lowering=False, debug=False)

    # DRAM layouts are pre-swizzled on the host to exactly match the SBUF
    # destination layout of each staged load, so every DMA is a single
    # fully-contiguous block (the naive row-major views load at ~130-230
    # GB/s because of 512B strided runs; these run at full bandwidth).
    qT_d = nc.dram_tensor("qT", [128, ET, T], BF16, kind="ExternalInput").ap()
    kT_d = nc.dram_tensor("kT", [128, 4, ET, 512], BF16,
                          kind="ExternalInput").ap()
    vT_d = nc.dram_tensor("vT", [128, 8, ET, 256], BF16,
                          kind="ExternalInput").ap()
    wqT_d = nc.dram_tensor("wqT", [128, 4, ET, 256], BF16,
                           kind="ExternalInput").ap()
    wkT_d = nc.dram_tensor("wkT", [128, ET, KVE], BF16,
                           kind="ExternalInput").ap()
    wvT_d = nc.dram_tensor("wvT", [128, ET, KVE], BF16,
                           kind="ExternalInput").ap()
    bq_d = nc.dram_tensor("bq", [128, 8], F32, kind="ExternalInput").ap()
    bk_d = nc.dram_tensor("bk", [128, 2], F32, kind="ExternalInput").ap()
    bv_d = nc.dram_tensor("bv", [KVE], F32, kind="ExternalInput").ap()
    gam_d = nc.dram_tensor("gam", [E], F32, kind="ExternalInput").ap()
    bet_d = nc.dram_tensor("bet", [E], F32, kind="ExternalInput").ap()
    out_d = nc.dram_tensor("out", [T, E], F32, kind="ExternalOutput").ap()

    def bcast_row(dram_ap, n):
        return bass.AP(tensor=dram_ap.tensor, offset=0, ap=[[0, n]] + dram_ap.ap)

    with tile.TileContext(nc) as tc:
        with tc.tile_pool(name="persist", bufs=1) as P:
            # -------- bulk loads, in exact consumption order -----------------
            queryT = P.tile([128, ET, T], BF16)
            wqT = P.tile([128, 4, ET, 256], BF16)
            keyT = P.tile([128, 4, ET, 512], BF16)
            wkT = P.tile([128, ET, KVE], BF16)
            valueT = P.tile([128, 8, ET, 256], BF16)
            wvT = P.tile([128, ET, KVE], BF16)
            bq_s = P.tile([128, 8], F32)
            bk_s = P.tile([128, 2], F32)
            bvB = P.tile([128, KVE], F32)
            if apply_gb:
                gamB = P.tile([128, E], F32)
                betB = P.tile([128, E], F32)
            # Loads in consumption order; the q-projection prefix (1MB) is
            # first so PE work starts (and warms the clock gate) early.
            nc.sync.dma_start(out=bq_s, in_=bq_d)
            nc.sync.dma_start(out=wqT[:, 0, :, 0:128], in_=wqT_d[:, 0, :, 0:128])
            nc.sync.dma_start(out=queryT, in_=qT_d)
            nc.sync.dma_start(out=wqT[:, 0, :, 128:256],
                              in_=wqT_d[:, 0, :, 128:256])
            nc.sync.dma_start(out=bk_s, in_=bk_d)
            nc.sync.dma_start(out=wkT, in_=wkT_d)
            nc.sync.dma_start(out=keyT[:, 0], in_=kT_d[:, 0])
            nc.sync.dma_start(out=keyT[:, 1], in_=kT_d[:, 1])
            nc.sync.dma_start(out=bvB, in_=bcast_row(bv_d, 128))
            nc.sync.dma_start(out=wvT, in_=wvT_d)
            nc.sync.dma_start(out=valueT[:, 0], in_=vT_d[:, 0])
            nc.sync.dma_start(out=valueT[:, 1], in_=vT_d[:, 1])
            nc.sync.dma_start(out=keyT[:, 2], in_=kT_d[:, 2])
            nc.sync.dma_start(out=valueT[:, 2], in_=vT_d[:, 2])
            nc.sync.dma_start(out=wqT[:, 1], in_=wqT_d[:, 1])
            nc.sync.dma_start(out=valueT[:, 3], in_=vT_d[:, 3])
            nc.sync.dma_start(out=keyT[:, 3], in_=kT_d[:, 3])
            nc.sync.dma_start(out=valueT[:, 4], in_=vT_d[:, 4])
            nc.sync.dma_start(out=wqT[:, 2:4], in_=wqT_d[:, 2:4])
            nc.sync.dma_start(out=valueT[:, 5], in_=vT_d[:, 5])
            nc.sync.dma_start(out=valueT[:, 6], in_=vT_d[:, 6])
            nc.sync.dma_start(out=valueT[:, 7], in_=vT_d[:, 7])
            if apply_gb:
                nc.sync.dma_start(out=gamB, in_=bcast_row(gam_d, 128))
                nc.sync.dma_start(out=betB, in_=bcast_row(bet_d, 128))

            eps_t = P.tile([128, 1], F32)
            nc.vector.memset(eps_t, EPS)

            q_sb = P.tile([128, 8, T], BF16)
            STATS = P.tile([128, 2, QH, 6], F32)
            K_sb = P.tile([128, 2, SEQ], BF16)
            # V per (st, kv) is padded to TR columns: cols 0..HD-1 = V,
            # cols HD..TR-2 = 0 (so the O matmul emits genuine zero rows and
            # the 80-row output transpose needs no junk-row memset), col
            # TR-1 = 1 (softmax denominator accumulator row).
            V_sb = P.tile([128, ST, KVH, TR], BF16)
            OUT = P.tile([128, 2, E], F32)
            nc.vector.memset(V_sb[:, :, :, HD:TR], 0.0)
            nc.vector.memset(V_sb[:, :, :, TR - 1:TR], 1.0)

            with tc.tile_pool(name="psP", bufs=1, space="PSUM") as psP, \
                 tc.tile_pool(name="srA", bufs=1, space="PSUM") as srA, \
                 tc.tile_pool(name="srB", bufs=1, space="PSUM") as srB, \
                 tc.tile_pool(name="op", bufs=1, space="PSUM") as opp, \
                 tc.tile_pool(name="ering", bufs=6) as erp, \
                 tc.tile_pool(name="tail", bufs=3) as tlp, \
                 tc.tile_pool(name="tailT", bufs=6) as trp:

                # ---- projection emitters (filler work units) ----
                def kproj(c, j, pk=None):
                    if pk is None:
                        pk = psP.tile([128, 512], F32, tag="pp", name=f"pk{c}{j}")
                    for e in range(ET):
                        nc.tensor.matmul(
                            pk, wkT[:, e, 128 * c:128 * (c + 1)],
                            keyT[:, j, e, :],
                            start=(e == 0), stop=(e == ET - 1))
                    nc.vector.tensor_scalar_add(
                        out=K_sb[:, c, 512 * j:512 * (j + 1)], in0=pk,
                        scalar1=bk_s[:, c:c + 1])

                def qproj(t, pq=None):
                    if pq is None:
                        pq = psP.tile([128, 512], F32, tag="pp", name=f"pq{t}")
                    for e in range(ET):
                        nc.tensor.matmul(
                            pq[:, 0:T],
                            wqT[:, t // 2, e, 128 * (t % 2):128 * (t % 2) + 128],
                            queryT[:, e, :], start=(e == 0), stop=(e == ET - 1))
                    nc.vector.tensor_scalar_add(
                        out=q_sb[:, t, :], in0=pq[:, 0:T], scalar1=bq_s[:, t:t + 1])

                def vproj(st):
                    pv = psP.tile([128, 512], F32, tag="pp")
                    for e in range(ET):
                        nc.tensor.matmul(
                            pv[:, 0:KVE],
                            valueT[:, st // 2, e,
                                   128 * (st % 2):128 * (st % 2) + 128],
                            wvT[:, e, :], start=(e == 0), stop=(e == ET - 1))
                    nc.vector.tensor_add(
                        out=V_sb[:, st, :, 0:HD],
                        in0=pv[:, 0:KVE].rearrange("p (h d) -> p h d", h=KVH),
                        in1=bvB.rearrange("p (h d) -> p h d", h=KVH))

                # pre-stream projections: q tiles 0, 1 first (smallest DMA
                # prefix -> earliest PE start, warms the clock gate), then
                # K channel-tile 0 chunk 0; rotated across the not-yet-used
                # attention psum banks so the PE never stalls on a
                # single-buffer evacuation.
                srA_pre = srA.tile([128, 3, 512], F32, tag="sA", name="srA_pre")
                qproj(0, srA_pre[:, 0, :])
                qproj(1, srA_pre[:, 1, :])
                kproj(0, 0, srA_pre[:, 2, :])

                # filler queue, sort keys in slot units.  The PE queue is
                # strictly in-order, so a filler emitted before its DMA
                # supply arrives stalls every matmul behind it: keys are
                # placed at max(first-use slot - ~10, DMA-arrival slot).
                _VKEY = [6, 8, 9, 11, 13, 15, 17, 20,
                         22, 24, 26, 28, 30, 32, 34, 36]
                _fl = ([(_VKEY[st], ("v", st)) for st in range(ST)] +
                       [(3, ("k0", 1)), (12, ("k0", 2)), (22, ("k0", 3))] +
                       [(24, ("q", 2)), (25, ("q", 3))] +
                       [(48, ("q", 4)), (52, ("q", 5))] +
                       [(50, ("k1", 0)), (58, ("k1", 1)),
                        (66, ("k1", 2)), (74, ("k1", 3))] +
                       [(80, ("q", 6)), (84, ("q", 7))])
                fillers = [f for _, f in sorted(_fl, key=lambda x: x[0])]
                f_keys = [k for k, _ in sorted(_fl, key=lambda x: x[0])]
                f_pos = {f: i for i, f in enumerate(fillers)}
                f_idx = 0

                def run_filler():
                    nonlocal f_idx
                    kind, arg = fillers[f_idx]
                    f_idx += 1
                    if kind == "v":
                        vproj(arg)
                    elif kind == "k1":
                        kproj(1, arg)
                    elif kind == "k0":
                        kproj(0, arg)
                    else:
                        qproj(arg)

                def ensure(*needs):
                    # emit fillers up to and including every needed one
                    idxs = [f_pos[n] for n in needs if n in f_pos]
                    while idxs and f_idx <= max(idxs):
                        run_filler()

                def s_needs(s):
                    p, st, hi = slot_info(s)
                    c = (KV_HI[p] if hi else KV_LO[p]) // 2
                    return [("q", 2 * p), ("q", 2 * p + 1),
                            ("k0", st // 4) if c == 0 else ("k1", st // 4)]

                # ---- global attention slot stream ----
                # slot s = (p, st, hi): p = s // 32, st = (s % 32) // 2, hi = s % 2
                n_slots = 128
                groups = []
                i, size_a = 0, True
                while i < n_slots:
                    k = 3 if size_a else 2
                    groups.append(list(range(i, min(i + k, n_slots))))
                    i += k
                    size_a = not size_a

                def slot_info(s):
                    p, r = divmod(s, 32)
                    st, hi = divmod(r, 2)
                    return p, st, hi

                o_banks = {}

                def emit_s(grp, sp, ep_):
                    for i_, s in enumerate(grp):
                        p, st, hi = slot_info(s)
                        if not hi:
                            nc.tensor.matmul(
                                sp[:, i_, :],
                                K_sb[0:64, KV_LO[p] // 2, 128 * st:128 * (st + 1)],
                                q_sb[0:64, 2 * p:2 * p + 2, :],
                                start=True, stop=True, tile_position=(0, 0))
                        else:
                            nc.tensor.matmul(
                                sp[:, i_, :],
                                K_sb[64:128, KV_HI[p] // 2, 128 * st:128 * (st + 1)],
                                q_sb[64:128, 2 * p:2 * p + 2, :],
                                start=True, stop=True, tile_position=(64, 0))

                def emit_or(grp, ep_):
                    tails = []
                    for i_, s in enumerate(grp):
                        p, st, hi = slot_info(s)
                        key = (p, hi)
                        if key not in o_banks:
                            o_banks[key] = opp.tile(
                                [128, 512], F32, tag="ohi" if hi else "olo",
                                name=f"o_ps{p}_{hi}")
                        kv = KV_HI[p] if hi else KV_LO[p]
                        nc.tensor.matmul(
                            o_banks[key][0:TR, :], V_sb[:, st, kv, :],
                            ep_[:, i_, :], start=(st == 0), stop=(st == ST - 1))
                        if st == ST - 1 and hi:
                            tails.append(p)
                    for p in tails:
                        quad_tail(p)

                # Tail processing: at O completion, ONE psum->SBUF cast (frees
                # the O psum banks) and ONE batched DMA x-bar transpose
                # [80,512] -> [128,4,80] per (p,hi) (the transpose has ~1.1us
                # fixed cost, so batching 4 chunks is ~3x cheaper than 4
                # calls).  The DVE post-ops (reciprocal / normalize /
                # bn_stats) are deferred >= 2 groups so the DVE never waits
                # at its queue head on the in-flight transpose (that would
                # block later projection evacuations and stall the PE via
                # the psP WAR dependency).
                dq = []          # (due_round, emit_fn)
                pump_round = 0

                def quad_tail(p):
                    for hi in (0, 1):
                        o_ps = o_banks.pop((p, hi))
                        o_st = tlp.tile([TR, 512], BF16, tag="ost",
                                        name=f"o_st{p}_{hi}")
                        nc.vector.tensor_copy(out=o_st, in_=o_ps[0:TR, :])
                        ot4 = trp.tile([128, 4, TR], BF16, tag="ot",
                                       name=f"ot{p}_{hi}")
                        # the very last transpose goes on scalar's HWDGE
                        # queue (its exp stream is finished by then).
                        eng = nc.scalar if (p == 3 and hi == 1) else nc.sync
                        eng.dma_start(out=ot4, in_=o_st, transpose=True)
                        rec = trp.tile([128, 4], F32, tag="rec",
                                       name=f"rec{p}_{hi}")

                        def mk_rec(drain, ot4=ot4, rec=rec):
                            nc.vector.reciprocal(
                                out=rec, in_=ot4[:, :, TR - 1:TR])

                        dq.append((pump_round + 2, mk_rec))

                        def mk_unit(k, on_act, p=p, hi=hi, ot4=ot4, rec=rec):
                            tt, ch = k % 2, k // 2
                            h = (OH if hi else EH)[2 * p + ch]
                            if on_act:
                                # drain only: scalar engine's exp stream is
                                # done, let it do the normalize in parallel
                                # with the DVE's bn_stats
                                nc.scalar.activation(
                                    out=OUT[:, tt, HD * h:HD * (h + 1)],
                                    in_=ot4[:, k, 0:HD], func=AF.Copy,
                                    scale=rec[:, k:k + 1])
                            else:
                                nc.vector.tensor_scalar_mul(
                                    out=OUT[:, tt, HD * h:HD * (h + 1)],
                                    in0=ot4[:, k, 0:HD], scalar1=rec[:, k:k + 1])
                            nc.vector.bn_stats(
                                out=STATS[:, tt, h, :],
                                in_=OUT[:, tt, HD * h:HD * (h + 1)])

                        for k in range(4):
                            dq.append((pump_round + 2 + k // 2,
                                       lambda drain, k=k, f=mk_unit: f(k, drain)))

                def pump(n=3, drain=False):
                    nonlocal pump_round
                    cnt = 0
                    while dq and dq[0][0] <= pump_round and cnt < n:
                        dq.pop(0)[1](drain)
                        cnt += 1
                    pump_round += 1

                pending = []  # (grp, ep) awaiting O emission (3-group lag)
                slot_hi = 0   # highest slot emitted so far
                for gi, grp in enumerate(groups):
                    for s in grp:
                        ensure(*s_needs(s))
                    k = len(grp)
                    sp = (srA if k == 3 else srB).tile(
                        [128, k, 512], F32, tag="sA" if k == 3 else "sB")
                    ep_ = erp.tile([128, 3, 512], BF16, tag="e")
                    emit_s(grp, sp, ep_)
                    slot_hi = grp[-1]
                    # drain fillers whose slot key has come due
                    while f_idx < len(fillers) and f_keys[f_idx] <= slot_hi:
                        run_filler()
                    nc.scalar.activation(out=ep_[:, 0:k, :], in_=sp[:, 0:k, :],
                                         func=AF.Exp, scale=SCALE)
                    pending.append((grp, ep_))
                    if len(pending) > 3:
                        pgrp, pep = pending.pop(0)
                        ensure(*[("v", slot_info(s)[1]) for s in pgrp])
                        emit_or(pgrp, pep)
                    pump(4)
                while pending:
                    pgrp, pep = pending.pop(0)
                    ensure(*[("v", slot_info(s)[1]) for s in pgrp])
                    emit_or(pgrp, pep)
                    pump(4)
                while f_idx < len(fillers):
                    run_filler()
                while dq:
                    pump(10, drain=True)

            # ---------------- layernorm + store ----------------
            # rstd = (var+eps)^-1/2 entirely on the DVE (Quake bit-trick +
            # two Newton iterations; final rel err ~4e-6) -- avoids the
            # 1.28us sqrt activation-table load in the kernel tail.
            I32 = mybir.dt.int32
            with tc.tile_pool(name="ln", bufs=2) as lnp:
                mvs = []
                for tt in range(2):
                    mv = lnp.tile([128, 2], F32, tag="mv", name=f"mv{tt}")
                    nc.vector.bn_aggr(out=mv, in_=STATS[:, tt, :, :])
                    mvs.append(mv)
                vpk = lnp.tile([128, 2], F32, tag="vpk")
                for tt in range(2):
                    nc.vector.tensor_scalar_add(out=vpk[:, tt:tt + 1],
                                                in0=mvs[tt][:, 1:2],
                                                scalar1=EPS)
                yb = lnp.tile([128, 2], F32, tag="yb")
                nc.vector.tensor_scalar(out=yb[:, :].bitcast(I32),
                                        in0=vpk[:, :].bitcast(I32),
                                        scalar1=1, scalar2=None,
                                        op0=ALU.arith_shift_right)
                nc.vector.tensor_scalar(out=yb[:, :].bitcast(I32),
                                        in0=yb[:, :].bitcast(I32),
                                        scalar1=-1, scalar2=0x5f3759df,
                                        op0=ALU.mult, op1=ALU.add)
                # one Newton iteration brings the bit-trick seed to ~0.17%
                # max rel err -- a per-token output scale error well inside
                # the error budget.
                tN = lnp.tile([128, 2], F32, tag="tN")
                nc.vector.tensor_mul(out=tN, in0=yb, in1=yb)
                nc.vector.tensor_mul(out=tN, in0=tN, in1=vpk)
                nc.vector.tensor_scalar(out=tN, in0=tN,
                                        scalar1=-0.5, scalar2=1.5,
                                        op0=ALU.mult, op1=ALU.add)
                nc.vector.tensor_mul(out=yb, in0=yb, in1=tN)
                for tt in range(2):
                    y = lnp.tile([128, E], F32, tag="y", name=f"y{tt}")
                    nc.vector.tensor_scalar(out=y, in0=OUT[:, tt, :],
                                            scalar1=mvs[tt][:, 0:1],
                                            scalar2=yb[:, tt:tt + 1],
                                            op0=ALU.subtract, op1=ALU.mult)
                    if apply_gb:
                        z = lnp.tile([128, E], F32, tag="z", name=f"z{tt}")
                        nc.vector.tensor_mul(out=z, in0=y, in1=gamB)
                        nc.vector.tensor_add(out=z, in0=z, in1=betB)
                    else:
                        z = y
                    eng = nc.sync if tt == 0 else nc.scalar
                    eng.dma_start(out=out_d[128 * tt:128 * (tt + 1), :], in_=z)

    nc.compile()
    return nc


def _prep_inputs(query, key, value, Wq, bq, Wk, bk, Wv, bv, gamma, beta):
    bf = ml_dtypes.bfloat16
    query, key, value = np.asarray(query), np.asarray(key), np.asarray(value)
    Wq, Wk, Wv = np.asarray(Wq), np.asarray(Wk), np.asarray(Wv)
    bq, bk, bv = np.asarray(bq), np.asarray(bk), np.asarray(bv)
    # DRAM layouts pre-swizzled to match the SBUF destinations (see _build):
    #   x of shape [E, S] -> [128p, chunks, ET, chunk_cols]
    qT = query[0].T.astype(bf)                     # (E, 2048)
    kT = key[0].T.astype(bf)
    vT = value[0].T.astype(bf)
    wqT = Wq.T[:, COLPERM].astype(bf)              # (E, E)
    wkT = Wk.T.astype(bf)                          # (E, 256)
    wvT = Wv.T.astype(bf)

    def swz(x, n_chunks):
        # (E, S) -> (128, n_chunks, ET, S // n_chunks)
        e, s = x.shape
        cc = s // n_chunks
        return np.ascontiguousarray(
            x.reshape(ET, 128, n_chunks, cc).transpose(1, 2, 0, 3))

    def etile(x):
        # (E, C) -> (128, ET, C)
        e, c = x.shape
        return np.ascontiguousarray(
            x.reshape(ET, 128, c).transpose(1, 0, 2))

    bq_p = np.ascontiguousarray(bq[COLPERM].reshape(8, 128).T.astype(np.float32))
    bk_p = np.ascontiguousarray(bk.reshape(2, 128).T.astype(np.float32))
    common = {
        "kT": swz(kT, 4), "vT": swz(vT, 8), "wqT": swz(wqT, 4),
        "wkT": etile(wkT), "wvT": etile(wvT),
        "bq": bq_p, "bk": bk_p, "bv": np.asarray(bv, np.float32),
        "gam": np.asarray(gamma, np.float32), "bet": np.asarray(beta, np.float32),
    }
    in_maps = []
    for c in range(N_CORES):
        m = dict(common)
        m["qT"] = etile(qT[:, T * c:T * (c + 1)])
        in_maps.append(m)
    return in_maps


def run(inputs, trace=False):
    trivial_gb = (np.all(np.asarray(inputs["gamma"]) == 1.0)
                  and np.all(np.asarray(inputs["beta"]) == 0.0))
    key = ("nc", not trivial_gb)
    if key not in _CACHE:
        _CACHE[key] = _build(apply_gb=not trivial_gb)
    nc = _CACHE[key]
    in_maps = _prep_inputs(**inputs)
    res = bass_utils.run_bass_kernel_spmd(
        nc, in_maps, core_ids=list(range(N_CORES)), trace=trace)
    out = np.empty((1, SEQ, E), np.float32)
    for c in range(N_CORES):
        out[0, T * c:T * (c + 1), :] = res.results[c]["out"]
    return out, res


def kernel(**inputs):
    out, _ = run(inputs, trace=False)
    return out


# revision 32
# speedup vs baseline: 1.0069x; 1.0069x over previous
"""MultiHead GQA (16 q heads / 4 kv heads, E=1024, n=2048, b=1) on 8 TRN2 cores.

Strategy: shard the 2048 query positions across the 8 cores (256 each); every
core computes the full K/V projections for all 2048 key positions (replicating
that small matmul is far cheaper than any collective at this size), runs
attention + layernorm for its own 256 tokens, and writes its 256x1024 slice.

All heavy matmuls run in bf16 (fp32 PSUM accumulation).  The host pre-
transposes and pre-casts the operands so that every matmul contraction
dimension lands on SBUF partitions with no on-device transposes; the only
on-device transposes are small bf16 DMA X-bar transposes of the attention
output (80x128 per chunk) and of the softmax denominators.

The emission order software-pipelines PE: the attention stream is a single
sequence of (pair, s-tile) slot groups; each group's S matmuls are followed
by projection "filler" matmuls and the *previous* group's O/R matmuls, so
the (in-order) PE queue never blocks on the exp that gates O/R.

Load order is tuned so the q-projection (smallest prefix: 1MB) starts the PE
early and warms the HAM clock gate before the S stream begins; the layernorm
rstd is computed as exp(-0.5*ln(var+eps)) so only one activation table set
(natural_log_exp_and_others) is ever loaded.

RoPE in the reference is the identity for b=1 (seq index = batch index = 0,
so cos=1 / sin=0 exactly); it is therefore omitted.
"""

import numpy as np
import ml_dtypes

import concourse.bass as bass
import concourse.bacc as bacc
import concourse.tile as tile
from concourse import mybir
from concourse import bass_utils

F32 = mybir.dt.float32
BF16 = mybir.dt.bfloat16
AF = mybir.ActivationFunctionType
ALU = mybir.AluOpType

N_CORES = 8
E = 1024
QH = 16
KVH = 4
HD = 64
KVE = KVH * HD
SEQ = 2048
T = SEQ // N_CORES   # 256 query tokens per core
ST = SEQ // 128      # 16 key s-tiles
ET = E // 128        # 8 contraction e-tiles
EPS = 1e-5
SCALE = 1.0 / (HD ** 0.5)
TR = 80              # transpose row count (>= HD+1, multiple of 16)

# Head bookkeeping: q-proj channel-tile t packs head EH[t] in partitions 0-63
# and head OH[t] in partitions 64-127.  EH heads use even kv heads (0, 2),
# OH heads use odd kv heads (1, 3), which matches the natural K-proj layout
# (K channel-tile 0 = kv0|kv1, tile 1 = kv2|kv3) with no partition shifts.
EH = [0, 1, 2, 3, 8, 9, 10, 11]
OH = [4, 5, 6, 7, 12, 13, 14, 15]
KV_LO = [EH[2 * p] // 4 for p in range(4)]   # [0, 0, 2, 2]
KV_HI = [OH[2 * p] // 4 for p in range(4)]   # [1, 1, 3, 3]

COLPERM = np.concatenate(
    [np.r_[EH[t] * HD:(EH[t] + 1) * HD, OH[t] * HD:(OH[t] + 1) * HD]
     for t in range(8)])

_CACHE = {}


def _build(apply_gb=True):
    nc = bacc.Bacc("TRN2", target_bir_lowering=False, debug=False)

    # DRAM layouts are pre-swizzled on the host to exactly match the SBUF
    # destination layout of each staged load, so every DMA is a single
    # fully-contiguous block (the naive row-major views load at ~130-230
    # GB/s because of 512B strided runs; these run at full bandwidth).
    qT_d = nc.dram_tensor("qT", [128, ET, T], BF16, kind="ExternalInput").ap()
    kT_d = nc.dram_tensor("kT", [128, 4, ET, 512], BF16,
                          kind="ExternalInput").ap()
    vT_d = nc.dram_tensor("vT", [128, 8, ET, 256], BF16,
                          kind="ExternalInput").ap()
    wqT_d = nc.dram_tensor("wqT", [128, 4, ET, 256], BF16,
                           kind="ExternalInput").ap()
    wkT_d = nc.dram_tensor("wkT", [128, ET, KVE], BF16,
                           kind="ExternalInput").ap()
    wvT_d = nc.dram_tensor("wvT", [128, ET, KVE], BF16,
                           kind="ExternalInput").ap()
    bq_d = nc.dram_tensor("bq", [128, 8], F32, kind="ExternalInput").ap()
    bk_d = nc.dram_tensor("bk", [128, 2], F32, kind="ExternalInput").ap()
    bv_d = nc.dram_tensor("bv", [KVE], F32, kind="ExternalInput").ap()
    gam_d = nc.dram_tensor("gam", [E], F32, kind="ExternalInput").ap()
    bet_d = nc.dram_tensor("bet", [E], F32, kind="ExternalInput").ap()
    out_d = nc.dram_tensor("out", [T, E], F32, kind="ExternalOutput").ap()

    def bcast_row(dram_ap, n):
        return bass.AP(tensor=dram_ap.tensor, offset=0, ap=[[0, n]] + dram_ap.ap)

    with tile.TileContext(nc) as tc:
        with tc.tile_pool(name="persist", bufs=1) as P:
            # -------- bulk loads, in exact consumption order -----------------
            queryT = P.tile([128, ET, T], BF16)
            wqT = P.tile([128, 4, ET, 256], BF16)
            keyT = P.tile([128, 4, ET, 512], BF16)
            wkT = P.tile([128, ET, KVE], BF16)
            valueT = P.tile([128, 8, ET, 256], BF16)
            wvT = P.tile([128, ET, KVE], BF16)
            bq_s = P.tile([128, 8], F32)
            bk_s = P.tile([128, 2], F32)
            bvB = P.tile([128, KVE], F32)
            if apply_gb:
                gamB = P.tile([128, E], F32)
                betB = P.tile([128, E], F32)
            # Loads in consumption order; the q-projection prefix (1MB) is
            # first so PE work starts (and warms the clock gate) early.
            nc.sync.dma_start(out=bq_s, in_=bq_d)
            nc.sync.dma_start(out=wqT[:, 0, :, 0:128], in_=wqT_d[:, 0, :, 0:128])
            nc.sync.dma_start(out=queryT, in_=qT_d)
            nc.sync.dma_start(out=wqT[:, 0, :, 128:256],
                              in_=wqT_d[:, 0, :, 128:256])
            nc.sync.dma_start(out=bk_s, in_=bk_d)
            nc.sync.dma_start(out=wkT, in_=wkT_d)
            nc.sync.dma_start(out=keyT[:, 0], in_=kT_d[:, 0])
            nc.sync.dma_start(out=keyT[:, 1], in_=kT_d[:, 1])
            nc.sync.dma_start(out=bvB, in_=bcast_row(bv_d, 128))
            nc.sync.dma_start(out=wvT, in_=wvT_d)
            nc.sync.dma_start(out=valueT[:, 0], in_=vT_d[:, 0])
            nc.sync.dma_start(out=valueT[:, 1], in_=vT_d[:, 1])
            nc.sync.dma_start(out=keyT[:, 2], in_=kT_d[:, 2])
            nc.sync.dma_start(out=valueT[:, 2], in_=vT_d[:, 2])
            nc.sync.dma_start(out=wqT[:, 1], in_=wqT_d[:, 1])
            nc.sync.dma_start(out=valueT[:, 3], in_=vT_d[:, 3])
            nc.sync.dma_start(out=keyT[:, 3], in_=kT_d[:, 3])
            nc.sync.dma_start(out=valueT[:, 4], in_=vT_d[:, 4])
            nc.sync.dma_start(out=wqT[:, 2:4], in_=wqT_d[:, 2:4])
            nc.sync.dma_start(out=valueT[:, 5], in_=vT_d[:, 5])
            nc.sync.dma_start(out=valueT[:, 6], in_=vT_d[:, 6])
            nc.sync.dma_start(out=valueT[:, 7], in_=vT_d[:, 7])
            if apply_gb:
                nc.sync.dma_start(out=gamB, in_=bcast_row(gam_d, 128))
                nc.sync.dma_start(out=betB, in_=bcast_row(bet_d, 128))

            eps_t = P.tile([128, 1], F32)
            nc.vector.memset(eps_t, EPS)
            dmy = P.tile([128, 512], BF16)
            nc.vector.memset(dmy, 0.0)

            q_sb = P.tile([128, 8, T], BF16)
            # layernorm partial stats per (token-half, head PAIR) -- adjacent
            # heads are contiguous 128-col windows of OUT, so bn_stats runs
            # once per pair.
            STATS = P.tile([128, 2, QH // 2, 6], F32)
            K_sb = P.tile([128, 2, SEQ], BF16)
            # V per (st, kv) is padded to TR columns: cols 0..HD-1 = V,
            # cols HD..TR-2 = 0 (so the O matmul emits genuine zero rows and
            # the 80-row output transpose needs no junk-row memset), col
            # TR-1 = 1 (softmax denominator accumulator row).
            V_sb = P.tile([128, ST, KVH, TR], BF16)
            OUT = P.tile([128, 2, E], F32)
            nc.vector.memset(V_sb[:, :, :, HD:TR], 0.0)
            nc.vector.memset(V_sb[:, :, :, TR - 1:TR], 1.0)

            with tc.tile_pool(name="psP", bufs=1, space="PSUM") as psP, \
                 tc.tile_pool(name="srA", bufs=1, space="PSUM") as srA, \
                 tc.tile_pool(name="srB", bufs=1, space="PSUM") as srB, \
                 tc.tile_pool(name="op", bufs=1, space="PSUM") as opp, \
                 tc.tile_pool(name="ering", bufs=6) as erp, \
                 tc.tile_pool(name="tail", bufs=3) as tlp, \
                 tc.tile_pool(name="tailT", bufs=6) as trp:

                # ---- projection emitters (filler work units) ----
                def kproj(c, j, pk=None):
                    if pk is None:
                        pk = psP.tile([128, 512], F32, tag="pp", name=f"pk{c}{j}")
                    for e in range(ET):
                        nc.tensor.matmul(
                            pk, wkT[:, e, 128 * c:128 * (c + 1)],
                            keyT[:, j, e, :],
                            start=(e == 0), stop=(e == ET - 1))
                    nc.vector.tensor_scalar_add(
                        out=K_sb[:, c, 512 * j:512 * (j + 1)], in0=pk,
                        scalar1=bk_s[:, c:c + 1])

                def qproj(t, pq=None):
                    if pq is None:
                        pq = psP.tile([128, 512], F32, tag="pp", name=f"pq{t}")
                    for e in range(ET):
                        nc.tensor.matmul(
                            pq[:, 0:T],
                            wqT[:, t // 2, e, 128 * (t % 2):128 * (t % 2) + 128],
                            queryT[:, e, :], start=(e == 0), stop=(e == ET - 1))
                    nc.vector.tensor_scalar_add(
                        out=q_sb[:, t, :], in0=pq[:, 0:T], scalar1=bq_s[:, t:t + 1])

                def vproj(st):
                    pv = psP.tile([128, 512], F32, tag="pp")
                    for e in range(ET):
                        nc.tensor.matmul(
                            pv[:, 0:KVE],
                            valueT[:, st // 2, e,
                                   128 * (st % 2):128 * (st % 2) + 128],
                            wvT[:, e, :], start=(e == 0), stop=(e == ET - 1))
                    nc.vector.tensor_add(
                        out=V_sb[:, st, :, 0:HD],
                        in0=pv[:, 0:KVE].rearrange("p (h d) -> p h d", h=KVH),
                        in1=bvB.rearrange("p (h d) -> p h d", h=KVH))

                # Warm the PE HAM clock-gate while the first loads are on the
                # wire: ~14 dummy matmuls on a zeroed tile keep the PE busy
                # from ~7us so the real projections run at 2.4GHz instead of
                # spending their first 3.4us at the cold 1.2GHz rate.
                warm_ps = psP.tile([128, 512], F32, tag="pp", name="warm")
                for _ in range(14):
                    nc.tensor.matmul(warm_ps, dmy[:, 0:128], dmy,
                                     start=True, stop=True)

                # pre-stream projections: q tiles 0, 1 first (smallest DMA
                # prefix -> earliest PE start), then K channel-tile 0 chunk
                # 0; rotated across the not-yet-used attention psum banks so
                # the PE never stalls on a single-buffer evacuation.
                srA_pre = srA.tile([128, 3, 512], F32, tag="sA", name="srA_pre")
                qproj(0, srA_pre[:, 0, :])
                qproj(1, srA_pre[:, 1, :])
                kproj(0, 0, srA_pre[:, 2, :])

                # filler queue, sort keys in slot units.  The PE queue is
                # strictly in-order, so a filler emitted before its DMA
                # supply arrives stalls every matmul behind it: keys are
                # placed at max(first-use slot - ~10, DMA-arrival slot).
                _VKEY = [6, 8, 9, 11, 13, 15, 17, 20,
                         22, 24, 26, 28, 30, 32, 34, 36]
                _fl = ([(_VKEY[st], ("v", st)) for st in range(ST)] +
                       [(3, ("k0", 1)), (12, ("k0", 2)), (22, ("k0", 3))] +
                       [(24, ("q", 2)), (25, ("q", 3))] +
                       [(48, ("q", 4)), (52, ("q", 5))] +
                       [(50, ("k1", 0)), (58, ("k1", 1)),
                        (66, ("k1", 2)), (74, ("k1", 3))] +
                       [(80, ("q", 6)), (84, ("q", 7))])
                fillers = [f for _, f in sorted(_fl, key=lambda x: x[0])]
                f_keys = [k for k, _ in sorted(_fl, key=lambda x: x[0])]
                f_pos = {f: i for i, f in enumerate(fillers)}
                f_idx = 0

                def run_filler():
                    nonlocal f_idx
                    kind, arg = fillers[f_idx]
                    f_idx += 1
                    if kind == "v":
                        vproj(arg)
                    elif kind == "k1":
                        kproj(1, arg)
                    elif kind == "k0":
                        kproj(0, arg)
                    else:
                        qproj(arg)

                def ensure(*needs):
                    # emit fillers up to and including every needed one
                    idxs = [f_pos[n] for n in needs if n in f_pos]
                    while idxs and f_idx <= max(idxs):
                        run_filler()

                def s_needs(s):
                    p, st, hi = slot_info(s)
                    c = (KV_HI[p] if hi else KV_LO[p]) // 2
                    return [("q", 2 * p), ("q", 2 * p + 1),
                            ("k0", st // 4) if c == 0 else ("k1", st // 4)]

                # ---- global attention slot stream ----
                # slot s = (p, st, hi): p = s // 32, st = (s % 32) // 2, hi = s % 2
                n_slots = 128
                groups = []
                i, size_a = 0, True
                while i < n_slots:
                    k = 3 if size_a else 2
                    groups.append(list(range(i, min(i + k, n_slots))))
                    i += k
                    size_a = not size_a

                def slot_info(s):
                    p, r = divmod(s, 32)
                    st, hi = divmod(r, 2)
                    return p, st, hi

                o_banks = {}

                def emit_s(grp, sp, ep_):
                    for i_, s in enumerate(grp):
                        p, st, hi = slot_info(s)
                        if not hi:
                            nc.tensor.matmul(
                                sp[:, i_, :],
                                K_sb[0:64, KV_LO[p] // 2, 128 * st:128 * (st + 1)],
                                q_sb[0:64, 2 * p:2 * p + 2, :],
                                start=True, stop=True, tile_position=(0, 0))
                        else:
                            nc.tensor.matmul(
                                sp[:, i_, :],
                                K_sb[64:128, KV_HI[p] // 2, 128 * st:128 * (st + 1)],
                                q_sb[64:128, 2 * p:2 * p + 2, :],
                                start=True, stop=True, tile_position=(64, 0))

                def emit_or(grp, ep_):
                    tails = []
                    for i_, s in enumerate(grp):
                        p, st, hi = slot_info(s)
                        key = (p, hi)
                        if key not in o_banks:
                            o_banks[key] = opp.tile(
                                [128, 512], F32, tag="ohi" if hi else "olo",
                                name=f"o_ps{p}_{hi}")
                        kv = KV_HI[p] if hi else KV_LO[p]
                        nc.tensor.matmul(
                            o_banks[key][0:TR, :], V_sb[:, st, kv, :],
                            ep_[:, i_, :], start=(st == 0), stop=(st == ST - 1))
                        if st == ST - 1 and hi:
                            tails.append(p)
                    for p in tails:
                        quad_tail(p)

                # Tail processing: at O completion, ONE psum->SBUF cast (frees
                # the O psum banks) and ONE batched DMA x-bar transpose
                # [80,512] -> [128,4,80] per (p,hi) (the transpose has ~1.1us
                # fixed cost, so batching 4 chunks is ~3x cheaper than 4
                # calls).  The DVE post-ops (reciprocal / normalize /
                # bn_stats) are deferred >= 2 groups so the DVE never waits
                # at its queue head on the in-flight transpose (that would
                # block later projection evacuations and stall the PE via
                # the psP WAR dependency).
                dq = []          # (due_round, emit_fn)
                pump_round = 0

                def quad_tail(p):
                    for hi in (0, 1):
                        o_ps = o_banks.pop((p, hi))
                        o_st = tlp.tile([TR, 512], BF16, tag="ost",
                                        name=f"o_st{p}_{hi}")
                        nc.vector.tensor_copy(out=o_st, in_=o_ps[0:TR, :])
                        ot4 = trp.tile([128, 4, TR], BF16, tag="ot",
                                       name=f"ot{p}_{hi}")
                        # the very last transpose goes on scalar's HWDGE
                        # queue (its exp stream is finished by then).
                        eng = nc.scalar if (p == 3 and hi == 1) else nc.sync
                        eng.dma_start(out=ot4, in_=o_st, transpose=True)
                        rec = trp.tile([128, 4], F32, tag="rec",
                                       name=f"rec{p}_{hi}")

                        def mk_rec(drain, ot4=ot4, rec=rec):
                            nc.vector.reciprocal(
                                out=rec, in_=ot4[:, :, TR - 1:TR])

                        dq.append((pump_round + 2, mk_rec))

                        def mk_unit(k, on_act, p=p, hi=hi, ot4=ot4, rec=rec):
                            tt, ch = k % 2, k // 2
                            h = (OH if hi else EH)[2 * p + ch]
                            if on_act:
                                # drain only: scalar engine's exp stream is
                                # done, let it do the normalize in parallel
                                # with the DVE's bn_stats
                                nc.scalar.activation(
                                    out=OUT[:, tt, HD * h:HD * (h + 1)],
                                    in_=ot4[:, k, 0:HD], func=AF.Copy,
                                    scale=rec[:, k:k + 1])
                            else:
                                nc.vector.tensor_scalar_mul(
                                    out=OUT[:, tt, HD * h:HD * (h + 1)],
                                    in0=ot4[:, k, 0:HD], scalar1=rec[:, k:k + 1])
                            if ch == 1:
                                # both heads of the pair normalized: one
                                # bn_stats over the contiguous 128-col window
                                h0 = (OH if hi else EH)[2 * p]
                                nc.vector.bn_stats(
                                    out=STATS[:, tt, h0 // 2, :],
                                    in_=OUT[:, tt, HD * h0:HD * h0 + 128])

                        for k in range(4):
                            dq.append((pump_round + 2 + k // 2,
                                       lambda drain, k=k, f=mk_unit: f(k, drain)))

                def pump(n=3, drain=False):
                    nonlocal pump_round
                    cnt = 0
                    while dq and dq[0][0] <= pump_round and cnt < n:
                        dq.pop(0)[1](drain)
                        cnt += 1
                    pump_round += 1

                pending = []  # (grp, ep) awaiting O emission (3-group lag)
                slot_hi = 0   # highest slot emitted so far
                for gi, grp in enumerate(groups):
                    for s in grp:
                        ensure(*s_needs(s))
                    k = len(grp)
                    sp = (srA if k == 3 else srB).tile(
                        [128, k, 512], F32, tag="sA" if k == 3 else "sB")
                    ep_ = erp.tile([128, 3, 512], BF16, tag="e")
                    emit_s(grp, sp, ep_)
                    slot_hi = grp[-1]
                    # drain fillers whose slot key has come due
                    while f_idx < len(fillers) and f_keys[f_idx] <= slot_hi:
                        run_filler()
                    nc.scalar.activation(out=ep_[:, 0:k, :], in_=sp[:, 0:k, :],
                                         func=AF.Exp, scale=SCALE)
                    pending.append((grp, ep_))
                    if len(pending) > 3:
                        pgrp, pep = pending.pop(0)
                        ensure(*[("v", slot_info(s)[1]) for s in pgrp])
                        emit_or(pgrp, pep)
                    pump(4)
                while pending:
                    pgrp, pep = pending.pop(0)
                    ensure(*[("v", slot_info(s)[1]) for s in pgrp])
                    emit_or(pgrp, pep)
                    pump(4)
                while f_idx < len(fillers):
                    run_filler()
                while dq:
                    pump(10, drain=True)

            # ---------------- layernorm + store ----------------
            # rstd = (var+eps)^-1/2 entirely on the DVE (Quake bit-trick +
            # two Newton iterations; final rel err ~4e-6) -- avoids the
            # 1.28us sqrt activation-table load in the kernel tail.
            I32 = mybir.dt.int32
            with tc.tile_pool(name="ln", bufs=2) as lnp:
                mvs = []
                for tt in range(2):
                    mv = lnp.tile([128, 2], F32, tag="mv", name=f"mv{tt}")
                    nc.vector.bn_aggr(out=mv, in_=STATS[:, tt, :, :])
                    mvs.append(mv)
                vpk = lnp.tile([128, 2], F32, tag="vpk")
                for tt in range(2):
                    nc.vector.tensor_scalar_add(out=vpk[:, tt:tt + 1],
                                                in0=mvs[tt][:, 1:2],
                                                scalar1=EPS)
                yb = lnp.tile([128, 2], F32, tag="yb")
                nc.vector.tensor_scalar(out=yb[:, :].bitcast(I32),
                                        in0=vpk[:, :].bitcast(I32),
                                        scalar1=1, scalar2=None,
                                        op0=ALU.arith_shift_right)
                nc.vector.tensor_scalar(out=yb[:, :].bitcast(I32),
                                        in0=yb[:, :].bitcast(I32),
                                        scalar1=-1, scalar2=0x5f3759df,
                                        op0=ALU.mult, op1=ALU.add)
                # one Newton iteration brings the bit-trick seed to ~0.17%
                # max rel err -- a per-token output scale error well inside
                # the error budget.
                tN = lnp.tile([128, 2], F32, tag="tN")
                nc.vector.tensor_mul(out=tN, in0=yb, in1=yb)
                nc.vector.tensor_mul(out=tN, in0=tN, in1=vpk)
                nc.vector.tensor_scalar(out=tN, in0=tN,
                                        scalar1=-0.5, scalar2=1.5,
                                        op0=ALU.mult, op1=ALU.add)
                nc.vector.tensor_mul(out=yb, in0=yb, in1=tN)
                for tt in range(2):
                    y = lnp.tile([128, E], F32, tag="y", name=f"y{tt}")
                    nc.vector.tensor_scalar(out=y, in0=OUT[:, tt, :],
                                            scalar1=mvs[tt][:, 0:1],
                                            scalar2=yb[:, tt:tt + 1],
                                            op0=ALU.subtract, op1=ALU.mult)
                    if apply_gb:
                        z = lnp.tile([128, E], F32, tag="z", name=f"z{tt}")
                        nc.vector.tensor_mul(out=z, in0=y, in1=gamB)
                        nc.vector.tensor_add(out=z, in0=z, in1=betB)
                    else:
                        z = y
                    eng = nc.sync if tt == 0 else nc.scalar
                    eng.dma_start(out=out_d[128 * tt:128 * (tt + 1), :], in_=z)

    nc.compile()
    return nc


def _prep_inputs(query, key, value, Wq, bq, Wk, bk, Wv, bv, gamma, beta):
    bf = ml_dtypes.bfloat16
    query, key, value = np.asarray(query), np.asarray(key), np.asarray(value)
    Wq, Wk, Wv = np.asarray(Wq), np.asarray(Wk), np.asarray(Wv)
    bq, bk, bv = np.asarray(bq), np.asarray(bk), np.asarray(bv)
    # DRAM layouts pre-swizzled to match the SBUF destinations (see _build):
    #   x of shape [E, S] -> [128p, chunks, ET, chunk_cols]
    qT = query[0].T.astype(bf)                     # (E, 2048)
    kT = key[0].T.astype(bf)
    vT = value[0].T.astype(bf)
    wqT = Wq.T[:, COLPERM].astype(bf)              # (E, E)
    wkT = Wk.T.astype(bf)                          # (E, 256)
    wvT = Wv.T.astype(bf)

    def swz(x, n_chunks):
        # (E, S) -> (128, n_chunks, ET, S // n_chunks)
        e, s = x.shape
        cc = s // n_chunks
        return np.ascontiguousarray(
            x.reshape(ET, 128, n_chunks, cc).transpose(1, 2, 0, 3))

    def etile(x):
        # (E, C) -> (128, ET, C)
        e, c = x.shape
        return np.ascontiguousarray(
            x.reshape(ET, 128, c).transpose(1, 0, 2))

    bq_p = np.ascontiguousarray(bq[COLPERM].reshape(8, 128).T.astype(np.float32))
    bk_p = np.ascontiguousarray(bk.reshape(2, 128).T.astype(np.float32))
    common = {
        "kT": swz(kT, 4), "vT": swz(vT, 8), "wqT": swz(wqT, 4),
        "wkT": etile(wkT), "wvT": etile(wvT),
        "bq": bq_p, "bk": bk_p, "bv": np.asarray(bv, np.float32),
        "gam": np.asarray(gamma, np.float32), "bet": np.asarray(beta, np.float32),
    }
    in_maps = []
    for c in range(N_CORES):
        m = dict(common)
        m["qT"] = etile(qT[:, T * c:T * (c + 1)])
        in_maps.append(m)
    return in_maps


def run(inputs, trace=False):
    trivial_gb = (np.all(np.asarray(inputs["gamma"]) == 1.0)
                  and np.all(np.asarray(inputs["beta"]) == 0.0))
    key = ("nc", not trivial_gb)
    if key not in _CACHE:
        _CACHE[key] = _build(apply_gb=not trivial_gb)
    nc = _CACHE[key]
    in_maps = _prep_inputs(**inputs)
    res = bass_utils.run_bass_kernel_spmd(
        nc, in_maps, core_ids=list(range(N_CORES)), trace=trace)
    out = np.empty((1, SEQ, E), np.float32)
    for c in range(N_CORES):
        out[0, T * c:T * (c + 1), :] = res.results[c]["out"]
    return out, res


def kernel(**inputs):
    out, _ = run(inputs, trace=False)
    return out
